# revision 2
# baseline (speedup 1.0000x reference)
"""Trainium2 Bass kernel for nn_KernelAttention (RBF kernel attention).

Math (per batch b):
    q = qs @ Wq + bq            [Q, H]
    k = ks @ Wk + bk            [K, H]
    v = vs @ Wv + bv            [K, H]
    S[i,j]  = 2*q_i.k_j - |q_i|^2 - |k_j|^2   (= -|q_i - k_j|^2)
    attn    = exp(S / sqrt(H)) row-normalized          [Q, K]
    out     = attn @ v @ Wo + bo                       [Q, H]

Sharding: 8 cores = 4 batches x 2 q-halves (1024 q rows each), K/V replicated.

Device strategy (per core):
  - PE transposes raw inputs to feature-major layout, projects to
    qT/kT [H, seq] (float32r matmuls), builds "extended" operands
    qTe/kTe [66, seq] whose extra rows carry |q|^2 / |k|^2 / constants so a
    single 66-deep matmul emits S directly in PSUM.
  - ACT computes exp(S*0.125) out of PSUM with fused per-row accumulation
    (row sums), once in [q,k] layout (for the attn output) and once in
    [k,q] layout (operand for the context matmul, float32r).
  - ctx^T accumulates over k-tiles in PSUM via (v@Wv@Wo) [k,H] stationaries;
    a final PE transpose + scale by 1/rowsum + bias produces `out`.

Host pre-folds the tiny weight algebra: WvWo = Wv@Wo, bo2 = bo + bv@Wo,
bq2 = 2*bq (the q-side of the extended matmul carries a factor 2).
"""

import numpy as np

import concourse.bass as bass
import concourse.tile as tile
from concourse import bacc, mybir
import concourse.bass_utils as bass_utils
from concourse.masks import make_identity

F32 = mybir.dt.float32
F32R = mybir.dt.float32r
AF = mybir.ActivationFunctionType

P = 128          # partitions / tile edge
D = 128          # input feature dim
H = 64           # projected head dim
QC = 1024        # q rows per core (half a batch)
K = 2048         # keys per batch
QT = QC // P     # 8 q tiles per core
KT = K // P      # 16 k tiles
E = 97           # extended contraction: rows 0:64 data, 64 aux1, 65:96 zero, 96 aux2
B = 4
Q = 2048
N_CORES = 8

_CACHED_NC = None


def _build():
    nc = bacc.Bacc("TRN2", target_bir_lowering=False, debug=False,
                   num_devices=N_CORES)

    qs_i = nc.dram_tensor("qs_i", [QC, D], F32, kind="ExternalInput").ap()
    ks_i = nc.dram_tensor("ks_i", [K, D], F32, kind="ExternalInput").ap()
    vs_i = nc.dram_tensor("vs_i", [K, D], F32, kind="ExternalInput").ap()
    wq_i = nc.dram_tensor("wq_i", [D, H], F32, kind="ExternalInput").ap()
    wk_i = nc.dram_tensor("wk_i", [D, H], F32, kind="ExternalInput").ap()
    wvwo_i = nc.dram_tensor("wvwo_i", [D, H], F32, kind="ExternalInput").ap()
    bq2_i = nc.dram_tensor("bq2_i", [H], F32, kind="ExternalInput").ap()
    bk_i = nc.dram_tensor("bk_i", [H], F32, kind="ExternalInput").ap()
    bo2_i = nc.dram_tensor("bo2_i", [H], F32, kind="ExternalInput").ap()

    attn_o = nc.dram_tensor("attn_o", [QC, K], F32, kind="ExternalOutput").ap()
    out_o = nc.dram_tensor("out_o", [QC, H], F32, kind="ExternalOutput").ap()

    attn_t = attn_o.rearrange("(t p) k -> t p k", p=P)   # [QT, P, K]
    out_t = out_o.rearrange("(t p) h -> t p h", p=P)     # [QT, P, H]

    with tile.TileContext(nc) as tc:
        with (
            tc.tile_pool(name="persist", bufs=1) as pe,
            tc.tile_pool(name="apool", bufs=3) as apool,
            tc.tile_pool(name="atpool", bufs=3) as atpool,
            tc.tile_pool(name="opool", bufs=2) as opool,
            tc.tile_pool(name="ps_misc", bufs=2, space="PSUM") as ps_misc,
            tc.tile_pool(name="ps_s", bufs=2, space="PSUM") as ps_s,
            tc.tile_pool(name="ps_st", bufs=2, space="PSUM") as ps_st,
            tc.tile_pool(name="ps_ctx", bufs=1, space="PSUM") as ps_ctx,
        ):
            # ---------------- Phase A: weights, transposes, projections ----
            wq_f = pe.tile([D, H], F32)
            wk_f = pe.tile([D, H], F32)
            wvwo_f = pe.tile([D, H], F32)
            nc.sync.dma_start(wq_f[:], wq_i)
            nc.sync.dma_start(wk_f[:], wk_i)
            nc.sync.dma_start(wvwo_f[:], wvwo_i)
            wq_r = pe.tile([D, H], F32R)
            wk_r = pe.tile([D, H], F32R)
            wvwo_r = pe.tile([D, H], F32R)
            nc.vector.tensor_copy(wq_r[:], wq_f[:])
            nc.vector.tensor_copy(wk_r[:], wk_f[:])
            nc.vector.tensor_copy(wvwo_r[:], wvwo_f[:])

            bq2_t = pe.tile([H, 1], F32)
            bk_t = pe.tile([H, 1], F32)
            nc.sync.dma_start(bq2_t[:], bq2_i[:, None])
            nc.sync.dma_start(bk_t[:], bk_i[:, None])
            bo2_bc = pe.tile([P, H], F32)
            nc.sync.dma_start(bo2_bc[:], bo2_i[None, :].partition_broadcast(P))

            ident = pe.tile([P, P], F32)
            make_identity(nc, ident[:])

            ones_f = pe.tile([H, 1], F32)
            nc.vector.memset(ones_f[:], 1.0)
            ones_r = pe.tile([H, 1], F32R)
            nc.vector.tensor_copy(ones_r[:], ones_f[:])

            # raw inputs, seq-major
            q_in = pe.tile([P, QT, D], F32)
            k_in = pe.tile([P, KT, D], F32)
            v_in = pe.tile([P, KT, D], F32)
            nc.sync.dma_start(q_in[:], qs_i.rearrange("(t p) d -> p t d", p=P))
            nc.sync.dma_start(k_in[:], ks_i.rearrange("(t p) d -> p t d", p=P))
            nc.sync.dma_start(v_in[:], vs_i.rearrange("(t p) d -> p t d", p=P))

            # feature-major raw inputs via PE transpose
            qsT = pe.tile([P, QT, P], F32R)
            ksT = pe.tile([P, KT, P], F32R)
            vsT = pe.tile([P, KT, P], F32R)
            for t in range(QT):
                pt = ps_misc.tile([P, P], F32, tag="m")
                nc.tensor.transpose(pt[:], q_in[:, t], ident[:])
                nc.vector.tensor_copy(qsT[:, t], pt[:])
            for t in range(KT):
                pt = ps_misc.tile([P, P], F32, tag="m")
                nc.tensor.transpose(pt[:], k_in[:, t], ident[:])
                nc.vector.tensor_copy(ksT[:, t], pt[:])
            for t in range(KT):
                pt = ps_misc.tile([P, P], F32, tag="m")
                nc.tensor.transpose(pt[:], v_in[:, t], ident[:])
                nc.vector.tensor_copy(vsT[:, t], pt[:])

            # extended operands: rows 0:64 data, row 64 aux1, rows 65:96
            # zero, row 96 aux2 (single-partition writes are only legal at
            # partition bases 0/32/64/96, so the aux rows sit at 64 and 96)
            qTe = pe.tile([E, QC], F32R)
            kTe = pe.tile([E, K], F32R)
            zrows = pe.tile([32, K], F32)
            nc.vector.memset(zrows[:], 0.0)
            nc.vector.tensor_copy(qTe[H:H + 32, :], zrows[:, :QC])
            nc.vector.tensor_copy(kTe[H:H + 32, :], zrows[:])

            # qTe rows 0:64  = 2*(qs@Wq + bq)  -> scale=2, bias=2bq
            for c in range(QC // 512):
                pp = ps_misc.tile([H, 512], F32, tag="m")
                nc.tensor.matmul(pp[:], wq_r[:], qsT[:, 4 * c:4 * c + 4],
                                 start=True, stop=True)
                nc.scalar.activation(qTe[0:H, 512 * c:512 * (c + 1)], pp[:],
                                     AF.Identity, bias=bq2_t[:], scale=2.0)
            # kTe rows 0:64 = ks@Wk + bk
            for c in range(K // 512):
                pp = ps_misc.tile([H, 512], F32, tag="m")
                nc.tensor.matmul(pp[:], wk_r[:], ksT[:, 4 * c:4 * c + 4],
                                 start=True, stop=True)
                nc.scalar.activation(kTe[0:H, 512 * c:512 * (c + 1)], pp[:],
                                     AF.Identity, bias=bk_t[:], scale=1.0)

            # squares (qsq = (2q)^2 = 4q^2 ; ksq = k^2)
            qsq = pe.tile([H, QC], F32R)
            ksq = pe.tile([H, K], F32R)
            nc.scalar.activation(qsq[:], qTe[0:H, :], AF.Square)
            nc.scalar.activation(ksq[:], kTe[0:H, :], AF.Square)

            # column sums -> |.|^2 rows.  qTe row 64 = 4*q2 (paired with
            # kTe row 64 = -1/4) ; kTe row 65 = -k2 (paired with qTe row 65 = 1)
            for c in range(QC // 512):
                pq = ps_misc.tile([1, 512], F32, tag="m")
                nc.tensor.matmul(pq[:], ones_r[:], qsq[:, 512 * c:512 * (c + 1)],
                                 start=True, stop=True)
                nc.scalar.activation(qTe[H:H + 1, 512 * c:512 * (c + 1)], pq[:],
                                     AF.Identity)
            for c in range(K // 512):
                pk = ps_misc.tile([1, 512], F32, tag="m")
                nc.tensor.matmul(pk[:], ones_r[:], ksq[:, 512 * c:512 * (c + 1)],
                                 start=True, stop=True)
                nc.scalar.activation(kTe[96:97, 512 * c:512 * (c + 1)],
                                     pk[:], AF.Identity, scale=-1.0)
            rowq = pe.tile([1, QC], F32)
            nc.vector.memset(rowq[:], 1.0)
            nc.vector.tensor_copy(qTe[96:97, :], rowq[:])
            rowk = pe.tile([1, K], F32)
            nc.vector.memset(rowk[:], -0.25)
            nc.vector.tensor_copy(kTe[H:H + 1, :], rowk[:])

            # vwo[k, H] = vs @ (Wv@Wo), per k-tile
            vwo = pe.tile([P, KT, H], F32R)
            for t in range(KT):
                pv = ps_misc.tile([P, H], F32, tag="m")
                nc.tensor.matmul(pv[:], vsT[:, t], wvwo_r[:],
                                 start=True, stop=True)
                nc.vector.tensor_copy(vwo[:, t], pv[:])

            # ---------------- Phase B: S -> exp -> rowsum -> attn out ------
            rs_all = pe.tile([P, QT, 4], F32)
            recip = pe.tile([P, QT], F32)
            for qt in range(QT):
                a_t = apool.tile([P, K], F32)
                for c in range(K // 512):
                    sp = ps_s.tile([P, 512], F32, tag="s")
                    nc.tensor.matmul(sp[:], qTe[:, qt * P:(qt + 1) * P],
                                     kTe[:, 512 * c:512 * (c + 1)],
                                     start=True, stop=True)
                    nc.scalar.activation(a_t[:, 512 * c:512 * (c + 1)], sp[:],
                                         AF.Exp, scale=0.125,
                                         accum_out=rs_all[:, qt, c:c + 1])
                rsum = pe.tile([P, QT], F32, tag="rsum")
                nc.vector.reduce_sum(rsum[:, qt:qt + 1], rs_all[:, qt],
                                     axis=mybir.AxisListType.X)
                nc.vector.reciprocal(recip[:, qt:qt + 1], rsum[:, qt:qt + 1])
                nc.vector.tensor_scalar_mul(a_t[:], a_t[:], recip[:, qt:qt + 1])
                nc.sync.dma_start(attn_t[qt], a_t[:])

            # ---------------- Phase C: S^T -> exp -> ctx^T -----------------
            ctx_ps = ps_ctx.tile([H, QC], F32)
            for kt in range(KT):
                at_t = atpool.tile([P, QC], F32R)
                for qh in range(QC // 512):
                    stp = ps_st.tile([P, 512], F32, tag="st")
                    nc.tensor.matmul(stp[:], kTe[:, kt * P:(kt + 1) * P],
                                     qTe[:, 512 * qh:512 * (qh + 1)],
                                     start=True, stop=True)
                    nc.scalar.activation(at_t[:, 512 * qh:512 * (qh + 1)],
                                         stp[:], AF.Exp, scale=0.125)
                for qh in range(QC // 512):
                    nc.tensor.matmul(ctx_ps[:, 512 * qh:512 * (qh + 1)],
                                     vwo[:, kt],
                                     at_t[:, 512 * qh:512 * (qh + 1)],
                                     start=(kt == 0), stop=(kt == KT - 1))

            # ---------------- Phase D: out = ctx^T.T * recip + bo2 ---------
            ctxs = pe.tile([H, QC], F32)
            nc.vector.tensor_copy(ctxs[:], ctx_ps[:])
            for qt in range(QT):
                tp = ps_misc.tile([P, H], F32, tag="m")
                nc.tensor.transpose(tp[:], ctxs[:, qt * P:(qt + 1) * P],
                                    ident[:H, :H])
                o_t = opool.tile([P, H], F32)
                nc.vector.tensor_scalar_mul(o_t[:], tp[:], recip[:, qt:qt + 1])
                nc.vector.tensor_add(o_t[:], o_t[:], bo2_bc[:])
                nc.sync.dma_start(out_t[qt], o_t[:])

    nc.compile()
    return nc


def _get_nc():
    global _CACHED_NC
    if _CACHED_NC is None:
        _CACHED_NC = _build()
    return _CACHED_NC


def kernel(qs, ks, vs, Wq, bq, Wk, bk, Wv, bv, Wo, bo):
    qs = np.asarray(qs, np.float32)
    ks = np.asarray(ks, np.float32)
    vs = np.asarray(vs, np.float32)
    Wq = np.asarray(Wq, np.float32)
    bq = np.asarray(bq, np.float32)
    Wk = np.asarray(Wk, np.float32)
    bk = np.asarray(bk, np.float32)
    Wv = np.asarray(Wv, np.float32)
    bv = np.asarray(bv, np.float32)
    Wo = np.asarray(Wo, np.float32)
    bo = np.asarray(bo, np.float32)

    wvwo = np.ascontiguousarray(Wv @ Wo)
    bo2 = np.ascontiguousarray(bo + bv @ Wo)
    bq2 = np.ascontiguousarray(2.0 * bq)

    nc = _get_nc()
    in_maps = []
    for c in range(N_CORES):
        b, h = divmod(c, 2)
        in_maps.append({
            "qs_i": np.ascontiguousarray(qs[b, h * QC:(h + 1) * QC]),
            "ks_i": np.ascontiguousarray(ks[b]),
            "vs_i": np.ascontiguousarray(vs[b]),
            "wq_i": Wq, "wk_i": Wk, "wvwo_i": wvwo,
            "bq2_i": bq2, "bk_i": bk, "bo2_i": bo2,
        })
    res = bass_utils.run_bass_kernel_spmd(nc, in_maps,
                                          core_ids=list(range(N_CORES)))
    attn = np.empty((B, Q, K), np.float32)
    out = np.empty((B, Q, H), np.float32)
    for c in range(N_CORES):
        b, h = divmod(c, 2)
        attn[b, h * QC:(h + 1) * QC] = res.results[c]["attn_o"]
        out[b, h * QC:(h + 1) * QC] = res.results[c]["out_o"]
    return out, attn
